# revision 35
# baseline (speedup 1.0000x reference)
"""Trainium2 Bass kernel for nn_Encoder_Model_89369679495588.

Single-layer transformer encoder (B=8, S=1024, D=512, H=8, FF=2048) with
whole-tensor layer norms. Sharding: data-parallel over batch, one batch
element per NeuronCore (8 cores). The whole-tensor layer_norm couples the
batch dimension, so each core computes partial (sum, sumsq) and the cores
exchange them with a tiny AllReduce (2 floats) before applying the norm.

On-chip layout: activations are kept transposed ([d, s] with d on the
partition axis) so every weight matrix ([d_in, d_out]) is usable directly
as the stationary matmul operand and biases are per-partition vectors.
"""

import os
import sys

for _p in ("/opt/trn_rl_repo",):
    if os.path.isdir(_p) and _p not in sys.path:
        sys.path.insert(0, _p)

import numpy as np

import concourse.bacc as bacc
import concourse.mybir as mybir
import concourse.tile as tile
from concourse import bass_utils
from concourse.masks import make_identity

B, S, D, H, DK, FF = 8, 1024, 512, 8, 64, 2048
EPS = 1e-5
N_CORES = 8
NTOT = float(B * S * D)  # layer-norm population size (global)
SCALE = 1.0 / ((D / H) / 2.0)  # reference divides scores by d_k/2 = 32

F32 = mybir.dt.float32
F32R = mybir.dt.float32r
AX = mybir.AxisListType
ALU = mybir.AluOpType
AF = mybir.ActivationFunctionType

# --- tunables (exercised via test sweeps) ---
OPT_SQ_ENGINE = "act"    # "act" | "dve"
OPT_RELU_ENGINE = "act"  # "act" | "dve"
OPT_W_BUFS = 3
OPT_CE_BUFS = 1
OPT_CC_SHARED = False
OPT_LN_FOLD = True

DT = D // 128  # 4 d-tiles
ST = S // 128  # 8 s-tiles
SCH = S // 512  # 2 s-chunks of 512
FT = FF // 128  # 16 ff-tiles


def _ln_apply(nc, psum, fixed, ones_k1, ar_sb, bc_sb, tiles, eps_sb):
    """Given ar_sb[1,2] = global (sum, sumsq), apply (x-mu)/sqrt(var+eps)
    in place to the listed [128, S] tile APs."""
    mval = fixed.tile([1, 1], F32, name=f"mval_{nc.next_id()}", tag="lnscalar", bufs=4)
    e2 = fixed.tile([1, 1], F32, name=f"e2_{nc.next_id()}", tag="lnscalar", bufs=4)
    mu2 = fixed.tile([1, 1], F32, name=f"mu2_{nc.next_id()}", tag="lnscalar", bufs=4)
    var = fixed.tile([1, 1], F32, name=f"var_{nc.next_id()}", tag="lnscalar", bufs=4)
    sd = fixed.tile([1, 1], F32, name=f"sd_{nc.next_id()}", tag="lnscalar", bufs=4)
    rsd = fixed.tile([1, 1], F32, name=f"rsd_{nc.next_id()}", tag="lnscalar", bufs=4)
    nmr = fixed.tile([1, 1], F32, name=f"nmr_{nc.next_id()}", tag="lnscalar", bufs=4)
    scal2 = fixed.tile([1, 2], F32, name=f"scal2_{nc.next_id()}", tag="lnscal2", bufs=2)

    nc.vector.tensor_scalar_mul(mval[:], ar_sb[:, 0:1], 1.0 / NTOT)
    nc.vector.tensor_scalar_mul(e2[:], ar_sb[:, 1:2], 1.0 / NTOT)
    nc.vector.tensor_mul(mu2[:], mval[:], mval[:])
    nc.vector.tensor_sub(var[:], e2[:], mu2[:])
    nc.scalar.activation(sd[:], var[:], AF.Sqrt, bias=eps_sb[:])
    nc.vector.reciprocal(rsd[:], sd[:])
    nc.vector.tensor_mul(nmr[:], mval[:], rsd[:])
    nc.vector.tensor_scalar_mul(nmr[:], nmr[:], -1.0)
    nc.vector.tensor_copy(scal2[:, 0:1], rsd[:])
    nc.vector.tensor_copy(scal2[:, 1:2], nmr[:])

    # broadcast (rsd, -mu*rsd) to all 128 partitions via a K=1 matmul
    ps_b = psum.tile([128, 2], F32, name=f"psb_{nc.next_id()}", tag="w", bufs=OPT_W_BUFS)
    nc.tensor.matmul(ps_b[:], ones_k1[:], scal2[:], start=True, stop=True)
    bc = bc_sb
    nc.scalar.copy(bc[:], ps_b[:])

    for t in tiles:
        # x = (x * rsd) + (-mu*rsd), fused per-partition scalars
        nc.vector.tensor_scalar(
            t, t, bc[:, 0:1], bc[:, 1:2], op0=ALU.mult, op1=ALU.add
        )
    # callers may pass tiles=[] and fold the affine into downstream ops


def build_program(n_cores: int = N_CORES, collectives: bool = True):
    nc = bacc.Bacc(
        "TRN2", target_bir_lowering=False, debug=False, num_devices=n_cores
    )

    dat = nc.dram_tensor("data", [S, D], F32, kind="ExternalInput").ap()
    wq_d = nc.dram_tensor("Wq", [D, D], F32R, kind="ExternalInput").ap()
    bq_d = nc.dram_tensor("bq", [D], F32, kind="ExternalInput").ap()
    wk_d = nc.dram_tensor("Wk", [D, D], F32R, kind="ExternalInput").ap()
    bk_d = nc.dram_tensor("bk", [D], F32, kind="ExternalInput").ap()
    wv_d = nc.dram_tensor("Wv", [D, D], F32R, kind="ExternalInput").ap()
    bv_d = nc.dram_tensor("bv", [D], F32R, kind="ExternalInput").ap()
    wo_d = nc.dram_tensor("Wo", [D, D], F32R, kind="ExternalInput").ap()
    bo_d = nc.dram_tensor("bo", [D], F32, kind="ExternalInput").ap()
    w1_d = nc.dram_tensor("W1", [D, FF], F32R, kind="ExternalInput").ap()
    b1_d = nc.dram_tensor("b1", [FF], F32, kind="ExternalInput").ap()
    w2_d = nc.dram_tensor("W2", [FF, D], F32R, kind="ExternalInput").ap()
    b2_d = nc.dram_tensor("b2", [D], F32, kind="ExternalInput").ap()
    w1cs_d = nc.dram_tensor("w1cs", [FF], F32, kind="ExternalInput").ap()
    out_d = nc.dram_tensor("out", [S, D], F32, kind="ExternalOutput").ap()

    with tile.TileContext(nc) as tc:
        with nc.allow_low_precision(
            reason="float32r tiles are 4-byte fp32 in SBUF; PE reads them reduced"
        ):
            _body(
                nc, tc, n_cores, collectives,
                dat, wq_d, bq_d, wk_d, bk_d, wv_d, bv_d, wo_d, bo_d,
                w1_d, b1_d, w2_d, b2_d, w1cs_d, out_d,
            )
    nc.compile()
    return nc


def _body(
    nc, tc, n_cores, collectives,
    dat, wq_d, bq_d, wk_d, bk_d, wv_d, bv_d, wo_d, bo_d,
    w1_d, b1_d, w2_d, b2_d, w1cs_d, out_d,
):
    from contextlib import ExitStack

    with ExitStack() as st:
        fixed = st.enter_context(tc.tile_pool(name="fixed", bufs=1))
        psum = st.enter_context(tc.tile_pool(name="psum", bufs=1, space="PSUM"))
        dram = st.enter_context(tc.tile_pool(name="dram", bufs=1, space="DRAM"))

        # ---- constants ----
        ident = fixed.tile([128, 128], F32)
        make_identity(nc, ident[:])
        # f32r matmul operands must be produced by rounding instructions,
        # so constants are staged through an f32 memset + DVE copy.
        ones_k1f = fixed.tile([1, 128], F32)
        nc.vector.memset(ones_k1f[:], 1.0)
        ones_k1 = fixed.tile([1, 128], F32R)
        nc.vector.tensor_copy(ones_k1[:], ones_k1f[:])
        ones128 = fixed.tile([128, 1], F32)
        nc.vector.memset(ones128[:], 1.0)
        onecolf = fixed.tile([128, 64], F32)
        nc.vector.memset(onecolf[:], 1.0)
        onecol = fixed.tile([128, 64], F32R)
        nc.vector.tensor_copy(onecol[:], onecolf[:])


        bq_sb = fixed.tile([128, DT], F32)
        nc.sync.dma_start(bq_sb[:], bq_d.rearrange("(t p) -> p t", p=128))
        bk_sb = fixed.tile([128, DT], F32)
        nc.sync.dma_start(bk_sb[:], bk_d.rearrange("(t p) -> p t", p=128))
        bo_sb = fixed.tile([128, DT], F32)
        nc.sync.dma_start(bo_sb[:], bo_d.rearrange("(t p) -> p t", p=128))
        b1_sb = fixed.tile([128, FT], F32)
        nc.sync.dma_start(b1_sb[:], b1_d.rearrange("(t p) -> p t", p=128))
        b2_sb = fixed.tile([128, DT], F32)
        nc.sync.dma_start(b2_sb[:], b2_d.rearrange("(t p) -> p t", p=128))
        bv_sb = fixed.tile([1, D], F32R)
        nc.sync.dma_start(bv_sb[:], bv_d.rearrange("(a m) -> a m", a=1))
        w1cs_sb = fixed.tile([128, FT], F32)
        nc.sync.dma_start(w1cs_sb[:], w1cs_d.rearrange("(t p) -> p t", p=128))
        cvec = fixed.tile([128, FT], F32)

        eps_sb = fixed.tile([1, 1], F32)
        nc.vector.memset(eps_sb[:], EPS)
        cc_sb1 = fixed.tile([1, 8], F32)
        nc.vector.memset(cc_sb1[:], 0.0)
        cc_sb2 = fixed.tile([1, 8], F32)
        nc.vector.memset(cc_sb2[:], 0.0)
        ar1 = fixed.tile([1, 8], F32)
        ar2 = fixed.tile([1, 8], F32)
        bc_sb1 = fixed.tile([128, 2], F32)
        bc_sb2 = fixed.tile([128, 2], F32)
        s1a = fixed.tile([128, 8], F32)
        s2a = fixed.tile([128, 8], F32)
        s1b = fixed.tile([128, 8], F32)
        s2b = fixed.tile([128, 8], F32)
        stats2a = fixed.tile([128, 2], F32)
        stats2b = fixed.tile([128, 2], F32)

        sq_pool = st.enter_context(tc.tile_pool(name="sq", bufs=2))

        # W1 lives until the end of FFN1; loaded early so FFN1 starts promptly
        w1_pool = st.enter_context(tc.tile_pool(name="w1p", bufs=1))
        w1_sb = w1_pool.tile([128, DT, FF], F32R)

        # persistent activations
        y1_pool = st.enter_context(tc.tile_pool(name="y1", bufs=1))
        y1T = y1_pool.tile([128, DT, S], F32R)  # mha + data, later ln1 out
        # y2 lives from FFN2 to the output phase; right-side so it doesn't
        # sit under the attention-phase pool stack
        y2_pool = st.enter_context(tc.tile_pool(name="y2", bufs=1, side="right"))

        with ExitStack() as st_attn:
            wqkv_pool = st_attn.enter_context(tc.tile_pool(name="wqkv", bufs=1))
            wq_sb = wqkv_pool.tile([128, DT, D], F32R)
            wk_sb = wqkv_pool.tile([128, DT, D], F32R)
            wv_sb = wqkv_pool.tile([128, DT, D], F32R)
            wo_sb = wqkv_pool.tile([128, DT, D], F32R)
            data_pool = st_attn.enter_context(tc.tile_pool(name="datap", bufs=1))
            dataT = data_pool.tile([128, DT, S], F32R)

            ctx_pool = st_attn.enter_context(tc.tile_pool(name="ctxp", bufs=1))
            ctxT = ctx_pool.tile([128, DT, S], F32R)

            # ---- phase A: load data, transpose to [d, s] ----
            with tc.tile_pool(name="xstd", bufs=1) as xstd_pool:
                x_std = xstd_pool.tile([128, ST, D], F32)
                dat_r = dat.rearrange("(i p) d -> p i d", p=128)
                for i in range(ST):
                    nc.sync.dma_start(x_std[:, i, :], dat_r[:, i, :])
                for i in range(ST):
                    ps_t = psum.tile([128, 512], F32, name="ps_t", tag="w", bufs=OPT_W_BUFS)
                    for j in range(DT):
                        nc.tensor.transpose(
                            ps_t[:, 128 * j:128 * (j + 1)],
                            x_std[:, i, 128 * j:128 * (j + 1)],
                            ident[:],
                        )
                    nc.scalar.copy(
                        dataT[:, :, 128 * i:128 * (i + 1)],
                        ps_t.rearrange("p (j c) -> p j c", j=DT),
                    )

            with ExitStack() as st_qkv:
                qkv_pool = st_qkv.enter_context(tc.tile_pool(name="qkv", bufs=1))
                qT = qkv_pool.tile([128, DT, S], F32R)
                kT = qkv_pool.tile([128, DT, S], F32R)
                v65 = qkv_pool.tile([128, ST, H, 65], F32R)
                nc.vector.tensor_copy(
                    v65[:, :, :, 64], onecol.rearrange("p (i h) -> p i h", i=ST)
                )

                # ---- phase B: q/k projections (transposed), v (standard) ----
                nc.sync.dma_start(wq_sb[:], wq_d.rearrange("(t p) m -> p t m", p=128))
                nc.sync.dma_start(wk_sb[:], wk_d.rearrange("(t p) m -> p t m", p=128))
                nc.sync.dma_start(wv_sb[:], wv_d.rearrange("(t p) m -> p t m", p=128))
                nc.sync.dma_start(wo_sb[:], wo_d.rearrange("(t p) m -> p t m", p=128))
                nc.sync.dma_start(w1_sb[:], w1_d.rearrange("(t p) m -> p t m", p=128))
                for dst, w_sb, b_sb in ((qT, wq_sb, bq_sb), (kT, wk_sb, bk_sb)):
                    for m in range(DT):
                        for n in range(SCH):
                            ps = psum.tile([128, 512], F32, name="ps_qk", tag="w", bufs=OPT_W_BUFS)
                            for k in range(DT):
                                nc.tensor.matmul(
                                    ps[:],
                                    w_sb[:, k, 128 * m:128 * (m + 1)],
                                    dataT[:, k, 512 * n:512 * (n + 1)],
                                    start=(k == 0),
                                    stop=(k == DT - 1),
                                )
                            nc.vector.tensor_scalar_add(
                                dst[:, m, 512 * n:512 * (n + 1)], ps[:], b_sb[:, m:m + 1]
                            )

                for i in range(ST):
                    ps = psum.tile([128, 512], F32, name="ps_v", tag="w", bufs=OPT_W_BUFS)
                    for k in range(DT):
                        nc.tensor.matmul(
                            ps[:],
                            dataT[:, k, 128 * i:128 * (i + 1)],
                            wv_sb[:, k, :],
                            start=(k == 0),
                            stop=False,
                        )
                    nc.tensor.matmul(ps[:], ones_k1[:], bv_sb[:], start=False, stop=True)
                    nc.vector.tensor_copy(
                        v65[:, i, :, 0:64], ps.rearrange("p (h e) -> p h e", h=H)
                    )

                # ---- phase C: attention (transposed scores, 2-head row pack)
                # chunk-outer so Wo for chunk n can interleave with the next
                # chunk's (ACT-bound) softmax work on the PE.
                with tc.tile_pool(name="pT", bufs=3) as pT_pool, \
                     tc.tile_pool(name="recipp", bufs=1) as recip_pool:
                    idx = 0
                    for n_q in range(SCH):
                        for p in range(DT):  # head pair -> heads (2p, 2p+1)
                            ce = psum.tile([65, 512], F32, name="ce", tag="cc" if OPT_CC_SHARED else "ce", bufs=2 * OPT_CE_BUFS if OPT_CC_SHARED else OPT_CE_BUFS)
                            co = psum.tile([65, 512], F32, name="co", tag="cc" if OPT_CC_SHARED else "co", bufs=2 * OPT_CE_BUFS if OPT_CC_SHARED else OPT_CE_BUFS)
                            for i in range(ST):
                                ps_s = psum.tile(
                                    [128, 1024], F32, name="ps_s", tag="w", bufs=OPT_W_BUFS
                                )
                                nc.tensor.matmul(
                                    ps_s[:, 0:512],
                                    kT[0:64, p, 128 * i:128 * (i + 1)],
                                    qT[0:64, p, 512 * n_q:512 * (n_q + 1)],
                                    start=True,
                                    stop=True,
                                )
                                nc.tensor.matmul(
                                    ps_s[:, 512:1024],
                                    kT[64:128, p, 128 * i:128 * (i + 1)],
                                    qT[64:128, p, 512 * n_q:512 * (n_q + 1)],
                                    start=True,
                                    stop=True,
                                )
                                pT = pT_pool.tile([128, 1024], F32R, name="pT")
                                nc.scalar.activation(pT[:], ps_s[:], AF.Exp, scale=SCALE)
                                nc.tensor.matmul(
                                    ce[:],
                                    v65[:, i, 2 * p, :],
                                    pT[:, 0:512],
                                    start=(i == 0),
                                    stop=(i == ST - 1),
                                )
                                nc.tensor.matmul(
                                    co[:],
                                    v65[:, i, 2 * p + 1, :],
                                    pT[:, 512:1024],
                                    start=(i == 0),
                                    stop=(i == ST - 1),
                                )
                            # copy ctx out first so ce/co release early, then
                            # denominators -> broadcast -> normalize in place
                            dst = ctxT[:, p, 512 * n_q:512 * (n_q + 1)]
                            nc.vector.tensor_copy(dst[0:64, :], ce[0:64, :])
                            nc.vector.tensor_copy(dst[64:128, :], co[0:64, :])
                            recip_e = recip_pool.tile([1, 512], F32R, name="recip_e")
                            recip_o = recip_pool.tile([1, 512], F32R, name="recip_o")
                            nc.vector.reciprocal(recip_e[:], ce[64:65, :])
                            nc.vector.reciprocal(recip_o[:], co[64:65, :])
                            rb = psum.tile([128, 1024], F32, name="rb", tag="w", bufs=OPT_W_BUFS)
                            nc.tensor.matmul(
                                rb[:, 0:512], ones_k1[:], recip_e[:],
                                start=True, stop=True,
                            )
                            nc.tensor.matmul(
                                rb[:, 512:1024], ones_k1[:], recip_o[:],
                                start=True, stop=True,
                            )
                            nc.vector.tensor_mul(
                                dst[0:64, :], dst[0:64, :], rb[0:64, 0:512]
                            )
                            nc.vector.tensor_mul(
                                dst[64:128, :], dst[64:128, :], rb[64:128, 512:1024]
                            )
                        # Wo projection + bias + residual + LN1 partial stats
                        # for this chunk (interleaves with next chunk softmax)
                        n = n_q
                        for m in range(DT):
                            ps = psum.tile([128, 512], F32, name="ps_o", tag="w", bufs=OPT_W_BUFS)
                            for k in range(DT):
                                nc.tensor.matmul(
                                    ps[:],
                                    wo_sb[:, k, 128 * m:128 * (m + 1)],
                                    ctxT[:, k, 512 * n:512 * (n + 1)],
                                    start=(k == 0),
                                    stop=(k == DT - 1),
                                )
                            ysl = y1T[:, m, 512 * n:512 * (n + 1)]
                            nc.vector.scalar_tensor_tensor(
                                out=ysl,
                                in0=ps[:],
                                scalar=bo_sb[:, m:m + 1],
                                in1=dataT[:, m, 512 * n:512 * (n + 1)],
                                op0=ALU.add,
                                op1=ALU.add,
                                accum_out=s1a[:, idx:idx + 1],
                            )
                            sq = sq_pool.tile([128, 512], F32, name="sq")
                            nc.scalar.activation(
                                sq[:], ysl, AF.Square, accum_out=s2a[:, idx:idx + 1]
                            )
                            idx += 1

        # ---- LN1 (global): all-reduce (sum, sumsq) ----
        nc.vector.tensor_reduce(stats2a[:, 0:1], s1a[:], axis=AX.X, op=ALU.add)
        nc.vector.tensor_reduce(stats2a[:, 1:2], s2a[:], axis=AX.X, op=ALU.add)
        ps_st = psum.tile([1, 2], F32, name="ps_st", tag="w", bufs=OPT_W_BUFS)
        nc.tensor.matmul(ps_st[:], ones128[:], stats2a[:], start=True, stop=True)
        nc.vector.tensor_copy(cc_sb1[:, 0:2], ps_st[:])
        cc1_in = dram.tile([1, 8], F32)
        nc.sync.dma_start(cc1_in[:], cc_sb1[:])
        if collectives:
            cc1_out = dram.tile([1, 8], F32, addr_space="Shared")
            nc.gpsimd.collective_compute(
                "AllReduce",
                ALU.add,
                replica_groups=[list(range(n_cores))],
                ins=[cc1_in[:]],
                outs=[cc1_out[:]],
            )
            nc.sync.dma_start(ar1[:], cc1_out[:])
        else:
            nc.sync.dma_start(ar1[:], cc1_in[:])
        if not OPT_LN_FOLD:
            _ln_apply(
                nc, psum, fixed, ones_k1f, ar1, bc_sb1,
                [y1T[:, m, :] for m in range(DT)], eps_sb,
            )

        # ---- FFN ----
        with ExitStack() as st_ffn:
            w2_pool = st_ffn.enter_context(tc.tile_pool(name="w2p", bufs=1))
            w2_sb = w2_pool.tile([128, FT, D], F32R)
            nc.sync.dma_start(w2_sb[:], w2_d.rearrange("(t p) m -> p t m", p=128))
            ff_pool = st_ffn.enter_context(tc.tile_pool(name="ffp", bufs=1))
            ffT = ff_pool.tile([128, FT, S], F32R)
            y2T = y2_pool.tile([128, DT, S], F32)  # x1 + ffn, later ln2 out

            for f in range(FT):
                for n in range(SCH):
                    ps = psum.tile([128, 512], F32, name="ps_f1", tag="w", bufs=OPT_W_BUFS)
                    for k in range(DT):
                        nc.tensor.matmul(
                            ps[:],
                            w1_sb[:, k, 128 * f:128 * (f + 1)],
                            y1T[:, k, 512 * n:512 * (n + 1)],
                            start=(k == 0),
                            stop=(k == DT - 1),
                        )
                    if OPT_LN_FOLD:
                        # evacuate raw z to SBUF without waiting for the AR
                        nc.scalar.copy(ffT[:, f, 512 * n:512 * (n + 1)], ps[:])
                    else:
                        nc.scalar.activation(
                            ffT[:, f, 512 * n:512 * (n + 1)], ps[:], AF.Relu,
                            bias=b1_sb[:, f:f + 1],
                        )
            if OPT_LN_FOLD:
                # LN1 scalars emitted only now: their ACT ops (sqrt, bc copy)
                # wait on the AllReduce and must not head-of-line-block the
                # ffT evacuation copies on the ACT queue.
                # relu(W1^T(a*y1+b)+b1) = relu(a*(W1^T y1) + (b*colsum(W1)+b1))
                _ln_apply(nc, psum, fixed, ones_k1f, ar1, bc_sb1, [], eps_sb)
                nc.vector.scalar_tensor_tensor(
                    out=cvec[:], in0=w1cs_sb[:], scalar=bc_sb1[:, 1:2], in1=b1_sb[:],
                    op0=ALU.mult, op1=ALU.add,
                )
                # relu(a*z + c) once the AR-derived scalars exist
                for f in range(FT):
                    for n in range(SCH):
                        sl = ffT[:, f, 512 * n:512 * (n + 1)]
                        nc.scalar.activation(
                            sl, sl, AF.Relu,
                            bias=cvec[:, f:f + 1], scale=bc_sb1[:, 0:1],
                        )
                # materialize x1 = a*y1 + b in place (for the FFN2 residual)
                for m in range(DT):
                    nc.vector.tensor_scalar(
                        y1T[:, m, :], y1T[:, m, :],
                        bc_sb1[:, 0:1], bc_sb1[:, 1:2], op0=ALU.mult, op1=ALU.add,
                    )

            idx = 0
            for m in range(DT):
                for n in range(SCH):
                    ps = psum.tile([128, 512], F32, name="ps_f2", tag="w", bufs=OPT_W_BUFS)
                    for k in range(FT):
                        nc.tensor.matmul(
                            ps[:],
                            w2_sb[:, k, 128 * m:128 * (m + 1)],
                            ffT[:, k, 512 * n:512 * (n + 1)],
                            start=(k == 0),
                            stop=(k == FT - 1),
                        )
                    ysl = y2T[:, m, 512 * n:512 * (n + 1)]
                    nc.vector.scalar_tensor_tensor(
                        out=ysl,
                        in0=ps[:],
                        scalar=b2_sb[:, m:m + 1],
                        in1=y1T[:, m, 512 * n:512 * (n + 1)],
                        op0=ALU.add,
                        op1=ALU.add,
                        accum_out=s1b[:, idx:idx + 1],
                    )
                    sq = sq_pool.tile([128, 512], F32, name="sq")
                    if OPT_SQ_ENGINE == "act":
                        nc.scalar.activation(
                            sq[:], ysl, AF.Square, accum_out=s2b[:, idx:idx + 1]
                        )
                    else:
                        nc.vector.scalar_tensor_tensor(
                            out=sq[:], in0=ysl, scalar=0.0, in1=ysl,
                            op0=ALU.add, op1=ALU.mult,
                            accum_out=s2b[:, idx:idx + 1],
                        )
                    idx += 1

        # ---- LN2 (global) ----
        nc.vector.tensor_reduce(stats2b[:, 0:1], s1b[:], axis=AX.X, op=ALU.add)
        nc.vector.tensor_reduce(stats2b[:, 1:2], s2b[:], axis=AX.X, op=ALU.add)
        ps_st2 = psum.tile([1, 2], F32, name="ps_st2", tag="w", bufs=OPT_W_BUFS)
        nc.tensor.matmul(ps_st2[:], ones128[:], stats2b[:], start=True, stop=True)
        nc.vector.tensor_copy(cc_sb2[:, 0:2], ps_st2[:])
        cc2_in = dram.tile([1, 8], F32)
        nc.sync.dma_start(cc2_in[:], cc_sb2[:])
        if collectives:
            cc2_out = dram.tile([1, 8], F32, addr_space="Shared")
            nc.gpsimd.collective_compute(
                "AllReduce",
                ALU.add,
                replica_groups=[list(range(n_cores))],
                ins=[cc2_in[:]],
                outs=[cc2_out[:]],
            )
            nc.sync.dma_start(ar2[:], cc2_out[:])
        else:
            nc.sync.dma_start(ar2[:], cc2_in[:])
        if OPT_LN_FOLD:
            # LN2 scalars only -- affine applied during the output copy
            _ln_apply(nc, psum, fixed, ones_k1f, ar2, bc_sb2, [], eps_sb)
        else:
            _ln_apply(
                nc, psum, fixed, ones_k1f, ar2, bc_sb2,
                [y2T[:, m, :] for m in range(DT)], eps_sb,
            )

        # ---- output: transpose back to [s, d] and store ----
        with tc.tile_pool(name="outp", bufs=2) as out_pool:
            for i in range(ST):
                ps_o = psum.tile([128, 512], F32, name="ps_out", tag="w", bufs=OPT_W_BUFS)
                for m in range(DT):
                    nc.tensor.transpose(
                        ps_o[:, 128 * m:128 * (m + 1)],
                        y2T[:, m, 128 * i:128 * (i + 1)],
                        ident[:],
                    )
                o_std = out_pool.tile([128, D], F32, name="o_std")
                if OPT_LN_FOLD:
                    nc.scalar.activation(
                        o_std[:], ps_o[:], AF.Identity,
                        bias=bc_sb2[:, 1:2], scale=bc_sb2[:, 0:1],
                    )
                else:
                    nc.scalar.copy(o_std[:], ps_o[:])
                nc.sync.dma_start(out_d[128 * i:128 * (i + 1), :], o_std[:])


_CACHE = {}


def _get_program():
    if "nc" not in _CACHE:
        _CACHE["nc"] = build_program(N_CORES, True)
    return _CACHE["nc"]


def kernel(**inputs) -> np.ndarray:
    nc = _get_program()
    data = np.asarray(inputs["data"], dtype=np.float32)
    shared = {
        k: np.ascontiguousarray(np.asarray(inputs[k], dtype=np.float32))
        for k in (
            "Wq", "bq", "Wk", "bk", "Wv", "bv", "Wo", "bo", "W1", "b1", "W2", "b2"
        )
    }
    shared["w1cs"] = shared["W1"].sum(axis=0)
    in_maps = []
    for c in range(N_CORES):
        m = {"data": np.ascontiguousarray(data[c])}
        m.update(shared)
        in_maps.append(m)
    res = bass_utils.run_bass_kernel_spmd(nc, in_maps, core_ids=list(range(N_CORES)))
    return np.stack([res.results[c]["out"] for c in range(N_CORES)], axis=0)
